# revision 1
# baseline (speedup 1.0000x reference)
"""DefocusBlur on 8 NeuronCores (Trainium2, Bass/Tile).

Depthwise 17x17 disk-blur of images [32,3,512,512] f32, reflect-101 pad.

Sharding: pure data parallel over batch — 4 images (12 planes) per core.

Per-core algorithm (fp8 DoubleRow rewrite of the banded-matmul scheme):
the 2D conv is decomposed per kernel column j into a 1-D conv along H
(a PSUM-accumulated banded matmul, contraction over 128 padded input
rows) with the W-shift j applied as a free-axis offset into the
W-padded input tile. fp8 matmuls run in MatmulPerfMode.DoubleRow: each
instruction carries TWO [128]-contraction k-tiles (0.5 cycles/row — 4x
the fp32r MAC rate), so two kernel columns (two W-shifts of the same
tile, one strided 3-D AP) share one matmul. The mirror pair
col0+col16 is pre-summed on the (otherwise idle) vector engine into a
tail region of the input tile — same tile, so it pairs with col8 in
one DoubleRow AP — leaving 16 operands = 8 matmuls per 112-row block.

Numerics (rel-err budget ~5e-3 vs the 2e-2 gate): inputs are
quantized to fp8 e4m3 on the host with error-diffusion rounding along
W (the conv's windowed sums see ~3x less quantization error than
round-to-nearest); weights are scaled by 246 — placing the dominant
disk weight near the top of a binade, found by sweep — and quantized
with error feedback per band column; the 1/246 descale rides the
PSUM->SBUF copy on the scalar engine (Activation Copy with scale),
which narrows to fp16 (not bf16: 3 extra mantissa bits, same DMA
cost).

DMA instruction count is minimized (HWDGE is a serial ~640ns/DMA
resource): input loads and output stores are batched per supertile
(3-D APs; supertile sizes [2,8,...,8,6,1] — small first for a fast
pipeline fill, tiny last for a short drain tail); weights load in one
DMA. All 12 padded planes are processed as one flat 6336-row space
(banded weights are translation-invariant); block rows that fall in
the 16-row pad seams are computed but sliced off on the host. A
dummy-matmul stream warms the PE clock (p-state ramp) during the
initial DMA wait and chains gaplessly into the real matmuls.
"""
import dataclasses

import numpy as np

_RADIUS = 8
_B, _C, _H, _W = 32, 3, 512, 512
_NCORES = 8
_PLANES = (_B // _NCORES) * _C
_M = 112
_KIN = _M + 2 * _RADIUS
_HP = _H + 2 * _RADIUS
_WP = _W + 2 * _RADIUS

_GH = _PLANES * _HP            # 6336 flat padded rows per core
_NSTART = _GH - 2 * _RADIUS    # 6320 valid window starts
_NBLOCKS = (_NSTART + _M - 1) // _M   # 57
_SB = 8                        # max blocks per DMA supertile
_SUPERS = [2, 4, 6, 8, 8, 8, 8, 8, 4, 1]       # sums to 57; ramped start
_XROWS = (_NBLOCKS - 1) * _M + _KIN    # 6400 padded input rows
_OROWS = _NBLOCKS * _M                 # 6384 output rows (tail garbage)

_SLOT = 1040                   # 528 input cols + 512 pair-sum cols
_WSCALE = 246.0
_NWARM = 12

# operand pairs per DoubleRow matmul: offsets into a slot's 1040 cols.
# 528 = the DVE pair-sum s0 = x[:, 0:512] + x[:, 16:528] (band k0).
# band index for offset o: o == 528 -> 0 else min(o, 16 - o).
_PAIRS = [(1, 15), (2, 14), (3, 13), (4, 12), (5, 11), (6, 10), (7, 9),
          (528, 8)]


def _disk_kernel():
    L = np.arange(-8, 9)
    X, Y = np.meshgrid(L, L)
    disk = ((X ** 2 + Y ** 2) <= _RADIUS ** 2).astype(np.float32)
    disk /= disk.sum()
    x = np.arange(3, dtype=np.float32) - 1
    g = np.exp(-(x ** 2) / (2.0 * 0.5 ** 2))
    g /= g.sum()
    k2 = np.outer(g, g).astype(np.float32)
    p = np.pad(disk, 1, mode="reflect")
    out = np.zeros_like(disk)
    for i in range(3):
        for j in range(3):
            out += k2[i, j] * p[i : i + 17, j : j + 17]
    return out


def _fp8_quantize_column(col, f8dt):
    """Round a 17-tap column to fp8 with error-feedback so the column sum
    stays tight (keeps the DC gain of the blur accurate)."""
    out = np.zeros_like(col)
    carry = 0.0
    for i in range(col.shape[0]):
        want = col[i] + carry
        q = float(np.asarray(want, dtype=np.float32).astype(f8dt))
        out[i] = q
        carry = want - q
    return out


def _banded_weights():
    """[KIN, 8, 2, M] fp8: band for _PAIRS[g][i] at [:, g, i, :]."""
    import ml_dtypes

    f8 = ml_dtypes.float8_e4m3
    k2d = _disk_kernel().astype(np.float64) * _WSCALE
    cols = {}
    for j in range(9):
        cols[j] = _fp8_quantize_column(k2d[:, j], f8)
    w = np.zeros((_KIN, len(_PAIRS), 2, _M), np.float32)
    for g, (a, b) in enumerate(_PAIRS):
        for i, o in enumerate((a, b)):
            band = cols[0 if o == 528 else min(o, 16 - o)]
            for m in range(_M):
                w[m : m + 17, g, i, m] = band
    return np.ascontiguousarray(w.astype(f8))


def _quantize_diffuse(a, f8dt):
    """fp8-quantize along the last axis with 1-D error diffusion: windowed
    sums (what the conv computes) see only the boundary carries instead of
    289 independent rounding errors."""
    a = a.astype(np.float32)
    q = np.empty(a.shape, dtype=f8dt)
    carry = np.zeros(a.shape[:-1], np.float32)
    for w in range(a.shape[-1]):
        want = a[..., w] + carry
        qq = want.astype(f8dt)
        q[..., w] = qq
        carry = want - qq.astype(np.float32)
    return q


_NC_CACHE = []


def _build_program(supers=None, nwarm=None, w_pool_ring=False, store_per=2,
                   outp_bufs=4):
    import concourse.bacc as bacc
    import concourse.mybir as mybir
    import concourse.tile as tile

    supers = supers or _SUPERS
    nwarm = nwarm or _NWARM
    f32 = mybir.dt.float32
    f8 = mybir.dt.float8e4
    f16 = mybir.dt.float16
    DR = mybir.MatmulPerfMode.DoubleRow
    ng = len(_PAIRS)
    assert sum(supers) == _NBLOCKS

    nc = bacc.Bacc("TRN2", target_bir_lowering=False, debug=False)
    x_d = nc.dram_tensor("x", [_XROWS, _WP], f8, kind="ExternalInput")
    w_d = nc.dram_tensor("w", [_KIN, ng, 2, _M], f8, kind="ExternalInput")
    o_d = nc.dram_tensor("o", [_OROWS, _W], f16, kind="ExternalOutput")

    with tile.TileContext(nc) as tc:
        with (
            tc.tile_pool(name="wpool", bufs=1) as wpool,
            tc.tile_pool(name="inp", bufs=3) as inp,
            tc.tile_pool(name="outp", bufs=outp_bufs) as outp,
            tc.tile_pool(name="ps", bufs=6, space="PSUM") as psp,
            tc.tile_pool(name="psw", bufs=1, space="PSUM") as psw,
        ):
            wt = wpool.tile([_KIN, ng, 2, _M], f8)
            # HAM warm-up: keep PE busy during the initial DMA wait so the
            # first real matmuls run at full clock.
            warm = wpool.tile([128, 64], f32)
            nc.gpsimd.memset(warm[:], 0.0)
            wps = psw.tile([64, 64], f32, tag="warm")
            for wi in range(nwarm):
                nc.tensor.matmul(
                    wps[:], warm[:, :64], warm[:, :64],
                    start=(wi == 0), stop=(wi == nwarm - 1),
                )
            # weights load first on the SP ring: they gate every matmul,
            # so they take the first serial HWDGE slot.
            if w_pool_ring:
                nc.gpsimd.dma_start(wt[:], w_d[:])
            else:
                nc.sync.dma_start(wt[:], w_d[:])
            sup_of = {}
            s0 = 0
            for n in supers:
                sup_of[s0] = n
                s0 += n
            sup_base = 0
            xt = None
            ot = None
            for b in range(_NBLOCKS):
                if b in sup_of:
                    nsup = sup_of[b]
                    sup_base = b
                    g0 = b * _M
                    xt = inp.tile([_KIN, _SB, _SLOT], f8, tag="xt")
                    # one DMA: nsup blocks of 128 rows, 112 rows apart
                    dst = xt[:, :nsup, : _WP]
                    src1 = x_d[g0 : g0 + _KIN, :]
                    src = dataclasses.replace(
                        src1,
                        ap=[list(src1.ap[0]), [_M * _WP, nsup], [1, _WP]],
                    )
                    nc.sync.dma_start(dst, src)
                i = b - sup_base
                slot = xt[:, i, :]
                # DVE pair-sum s0 = col0 + col16 into the slot tail
                nc.vector.tensor_add(
                    xt[:, i, _WP : _WP + _W],
                    xt[:, i, 0:_W],
                    xt[:, i, 2 * _RADIUS : 2 * _RADIUS + _W],
                )
                ps = psp.tile([_M, _W], f32, tag="ps")
                pdim = list(slot.ap[0])
                for g, (a, bb) in enumerate(_PAIRS):
                    rhs = dataclasses.replace(
                        slot,
                        offset=slot.offset + a,
                        ap=[pdim, [bb - a, 2], [1, _W]],
                    )
                    nc.tensor.matmul(
                        ps[:], wt[:, g, :, :], rhs,
                        start=(g == 0), stop=(g == ng - 1),
                        perf_mode=DR,
                    )
                sp = store_per
                if b % sp == 0:
                    ot = outp.tile([_M, sp, _W], f16, tag="ot")
                if b == _NBLOCKS - 1:
                    # final block rides the drain tail: copy only valid rows
                    nv = _NSTART - b * _M
                    nc.scalar.mul(
                        ot[:nv, b % sp, :], ps[:nv, :], 1.0 / _WSCALE
                    )
                else:
                    nc.scalar.mul(ot[:, b % sp, :], ps[:], 1.0 / _WSCALE)
                # store every `sp` blocks (output rows are block-major), so
                # transfers pipeline on the serial DMA-engines slot instead
                # of bunching behind a whole supertile's copies.
                ns = b % sp + 1
                if ns == sp or b == _NBLOCKS - 1:
                    bb0 = b - ns + 1
                    if b == _NBLOCKS - 1:
                        # final block: only 48 of its rows are valid starts
                        nv = _NSTART - b * _M
                        nc.sync.dma_start(
                            o_d[b * _M : b * _M + nv, :],
                            ot[:nv, ns - 1, :],
                        )
                        ns -= 1
                    if ns > 0:
                        osrc = ot[:, :ns, :]
                        od1 = o_d[bb0 * _M : (bb0 + 1) * _M, :]
                        odst = dataclasses.replace(
                            od1,
                            ap=[list(od1.ap[0]), [_M * _W, ns], [1, _W]],
                        )
                        nc.sync.dma_start(odst, osrc)
    nc.compile()
    return nc


def _get_program():
    if not _NC_CACHE:
        _NC_CACHE.append(_build_program())
    return _NC_CACHE[0]


def kernel(images: np.ndarray) -> np.ndarray:
    import ml_dtypes
    from concourse.bass_utils import run_bass_kernel_spmd

    f8 = ml_dtypes.float8_e4m3
    images = np.asarray(images, dtype=np.float32)
    padded = np.pad(
        images, ((0, 0), (0, 0), (_RADIUS, _RADIUS), (_RADIUS, _RADIUS)),
        mode="reflect",
    )
    shards = padded.reshape(_NCORES, _PLANES * _HP, _WP)
    xs = np.zeros((_NCORES, _XROWS, _WP), dtype=f8)
    xs[:, : _PLANES * _HP, :] = _quantize_diffuse(shards, f8)
    w = _banded_weights()
    nc = _get_program()
    in_maps = [{"x": xs[c], "w": w} for c in range(_NCORES)]
    res = run_bass_kernel_spmd(nc, in_maps, list(range(_NCORES)))
    out = np.stack(
        [np.asarray(res.results[c]["o"]) for c in range(_NCORES)], axis=0
    )
    out = out[:, : _PLANES * _HP, :].astype(np.float32)
    out = out.reshape(_NCORES, _PLANES, _HP, _W)[:, :, : _H, :]
    return np.ascontiguousarray(
        out.reshape(_B, _C, _H, _W).astype(np.float32)
    )



# revision 46
# speedup vs baseline: 1.0293x; 1.0293x over previous
"""DefocusBlur on 8 NeuronCores (Trainium2, Bass/Tile).

Depthwise 17x17 disk-blur of images [32,3,512,512] f32, reflect-101 pad.

Sharding: pure data parallel over batch — 4 images (12 planes) per core.

Per-core algorithm (fp8 DoubleRow banded-matmul scheme, EXACT 17-column
kernel): the 2D conv is decomposed per kernel column j into a 1-D conv
along H (a PSUM-accumulated banded matmul, contraction over 128 padded
input rows) with the W-shift j applied as a free-axis offset into the
W-padded input tile. fp8 matmuls run in MatmulPerfMode.DoubleRow: each
instruction carries TWO [128]-contraction k-tiles (0.5 cycles/row), so
two kernel columns (two W-shifts of the same tile, one strided 3-D AP)
share one matmul.

Pass reduction via mirror-column pre-sums (the kernel is left-right
symmetric, K[:,j] == K[:,16-j], so x(j)+x(16-j) consumes one band
instead of two): every block gets s1 = x(0)+x(16) on the DVE (17 -> 16
operands -> 8 DR matmuls); ODD blocks additionally get s2 = x(1)+x(15)
and s3 = x(2)+x(14) on the Pool/GpSimd engine (17 -> 14 operands -> 7
DR matmuls). The Pool sums use InstTensorScalarPtr
(out = in0*1 + in1), whose software implementation prices ~30% cheaper
than plain adds, letting Pool sustain two sums per odd block under the
PE block budget. Net PE work: 15 DR matmuls per block pair instead
of 16.

Numerics (rel-err ~3e-3 vs the 2e-2 gate): inputs are quantized to
fp8 e4m3 on the host with error-diffusion rounding along W (the conv's
windowed sums see ~3x less quantization error than round-to-nearest);
weights are scaled by 246 — placing the dominant disk weight near the
top of a binade, found by sweep — and quantized with error feedback
per band column; the 1/246 descale rides the PSUM->SBUF copy on the
scalar engine (Activation Copy with scale), which narrows to fp16.

Engine schedule per pair of blocks: 15 DR matmuls (PE), 2 DVE sums +
2 Pool sums, ONE batched Activation descale-copy [112,2,512] (PSUM
tiles span 2 banks; batching halves the per-instr fixed cost), output
DMA per 4 blocks. Sum-consuming matmuls are ordered LAST in each block
so the vector engines run a block ahead without stalling the PE. DMA
instruction count is minimized (HWDGE is a serial ~640ns/DMA
resource): input loads are batched per supertile (3-D APs, ramped
sizes for a fast pipeline fill), weights load in two DMAs (the first
block's bands first, so the PE can start early). All 12 padded planes
are processed as one flat 6336-row space (banded weights are
translation-invariant); block rows in the 16-row pad seams are
computed but sliced off on the host. A dummy-matmul stream warms the
PE clock (p-state ramp) during the initial DMA wait and chains
gaplessly into the real matmuls.
"""
import dataclasses

import numpy as np

_RADIUS = 8
_B, _C, _H, _W = 32, 3, 512, 512
_NCORES = 8
_PLANES = (_B // _NCORES) * _C
_M = 112
_KIN = _M + 2 * _RADIUS
_HP = _H + 2 * _RADIUS
_WP = _W + 2 * _RADIUS

_GH = _PLANES * _HP            # 6336 flat padded rows per core
_NSTART = _GH - 2 * _RADIUS    # 6320 valid window starts
_NBLOCKS = (_NSTART + _M - 1) // _M   # 57
_SB = 8                        # max blocks per DMA supertile
# per-POSITION supertile sizes; position 0 is the tail block (56),
# computed FIRST so its copy/store are off the drain-critical path.
_SUPERS = [1, 1, 2, 4, 8, 8, 8, 8, 8, 8, 1]    # sums to 57; ramped start
_XROWS = (_NBLOCKS - 1) * _M + _KIN    # 6400 padded input rows
_OROWS = _NBLOCKS * _M                 # 6384 output rows (tail garbage)

# slot layout: [raw 528 | s1 512 | s2 512 | s3 512]
_S1 = _WP
_S2 = _WP + _W
_S3 = _WP + 2 * _W
_SLOT = _WP + 3 * _W           # 2064
_WSCALE = 246.0
_NWARM = 1

# operand pairs per DoubleRow matmul: offsets into a slot's 2064 cols.
# s-offsets (>=528) refer to device pre-sums of mirror columns.
# Sum-consuming pairs go LAST so early matmuls don't wait on DVE/Pool.
_PAIRS_EVEN = [(1, 15), (2, 14), (3, 13), (4, 12), (5, 11), (6, 10),
               (7, 9), (_S1, 8)]                       # 8 DR
_PAIRS_ODD = [(3, 13), (4, 12), (5, 11), (6, 10), (7, 9),
              (_S2, _S3), (_S1, 8)]                    # 7 DR
_NGE = len(_PAIRS_EVEN)
_NGO = len(_PAIRS_ODD)
_NG = _NGE + _NGO              # 15 weight groups


def _is_odd_block(b):
    """7-DR (triple-sum) blocks: odd blocks 3..53. Blocks 0-2 stay 8-DR
    so the PE can start on the first weight-group DMA alone; the tail
    block (computed FIRST, before the odd weight groups land) stays
    8-DR."""
    return b % 2 == 1 and 7 <= b <= 55 and ((b - 1) // 2) % 3 != 2


def _s2_engine_is_dve(b):
    """Rebalance: DVE (~15us slack) takes s2 for every other 7-DR block,
    keeping Pool well under the PE block budget."""
    return (b // 2) % 2 == 0


def _store_group(b):
    """(group_start, group_size) for output stores: groups of 4 through
    block 51, a pair (52-53), then singles so the final DMAs on the
    drain tail are small and issue early."""
    if b <= 51:
        return b - b % 4, 4
    if b <= 53:
        return 52, 2
    if b <= 55:
        return 54, 2
    return b, 1


def _disk_kernel():
    L = np.arange(-8, 9)
    X, Y = np.meshgrid(L, L)
    disk = ((X ** 2 + Y ** 2) <= _RADIUS ** 2).astype(np.float32)
    disk /= disk.sum()
    x = np.arange(3, dtype=np.float32) - 1
    g = np.exp(-(x ** 2) / (2.0 * 0.5 ** 2))
    g /= g.sum()
    k2 = np.outer(g, g).astype(np.float32)
    p = np.pad(disk, 1, mode="reflect")
    out = np.zeros_like(disk)
    for i in range(3):
        for j in range(3):
            out += k2[i, j] * p[i : i + 17, j : j + 17]
    return out


def _fp8_quantize_column(col, f8dt):
    """Round a 17-tap column to fp8 with error-feedback so the column sum
    stays tight (keeps the DC gain of the blur accurate)."""
    out = np.zeros_like(col)
    carry = 0.0
    for i in range(col.shape[0]):
        want = col[i] + carry
        q = float(np.asarray(want, dtype=np.float32).astype(f8dt))
        out[i] = q
        carry = want - q
    return out


def _band_col(o):
    """Kernel column index for operand offset o (s-offsets map to the
    mirror-class column their pre-sum carries)."""
    if o == _S1:
        return 0
    if o == _S2:
        return 1
    if o == _S3:
        return 2
    return min(o, 16 - o)


def _banded_weights():
    """[KIN, NG, 2, M] fp8: groups 0..7 = even-block pairs, 8..14 = odd."""
    import ml_dtypes

    f8 = ml_dtypes.float8_e4m3
    k2d = _disk_kernel().astype(np.float64) * _WSCALE
    cols = {}
    for j in range(9):
        cols[j] = _fp8_quantize_column(k2d[:, j], f8)
    w = np.zeros((_KIN, _NG, 2, _M), np.float32)
    for g, (a, b) in enumerate(_PAIRS_EVEN + _PAIRS_ODD):
        for i, o in enumerate((a, b)):
            band = cols[_band_col(o)]
            for m in range(_M):
                w[m : m + 17, g, i, m] = band
    return np.ascontiguousarray(w.astype(f8))


def _quantize_diffuse(a, f8dt):
    """fp8-quantize along the last axis with 1-D error diffusion: windowed
    sums (what the conv computes) see only the boundary carries instead of
    289 independent rounding errors."""
    a = a.astype(np.float32)
    q = np.empty(a.shape, dtype=f8dt)
    carry = np.zeros(a.shape[:-1], np.float32)
    for w in range(a.shape[-1]):
        want = a[..., w] + carry
        qq = want.astype(f8dt)
        q[..., w] = qq
        carry = want - qq.astype(np.float32)
    return q


_NC_CACHE = []


def _build_program(supers=None, nwarm=None, store_per=4, outp_bufs=6):
    import concourse.bacc as bacc
    import concourse.mybir as mybir
    import concourse.tile as tile

    supers = supers or _SUPERS
    nwarm = nwarm or _NWARM
    f32 = mybir.dt.float32
    f8 = mybir.dt.float8e4
    f16 = mybir.dt.float16
    DR = mybir.MatmulPerfMode.DoubleRow
    MULT = mybir.AluOpType.mult
    ADD = mybir.AluOpType.add
    assert sum(supers) == _NBLOCKS

    nc = bacc.Bacc("TRN2", target_bir_lowering=False, debug=False)
    x_d = nc.dram_tensor("x", [_XROWS, _WP], f8, kind="ExternalInput")
    w_d = nc.dram_tensor("w", [_KIN, _NG, 2, _M], f8, kind="ExternalInput")
    o_d = nc.dram_tensor("o", [_OROWS, _W], f16, kind="ExternalOutput")

    with tile.TileContext(nc) as tc:
        with (
            tc.tile_pool(name="wpool", bufs=1) as wpool,
            tc.tile_pool(name="inp", bufs=4) as inp,
            tc.tile_pool(name="outp", bufs=outp_bufs) as outp,
            tc.tile_pool(name="ps", bufs=4, space="PSUM") as psp,
        ):
            wt = wpool.tile([_KIN, _NG, 2, _M], f8)
            # HAM warm-up: keep PE busy during the initial DMA wait so the
            # first real matmuls run at full clock. Memset on the (idle
            # until ~3.5us) DVE so the warm stream starts immediately.
            warm = wpool.tile([128, 64], f32)
            nc.vector.memset(warm[:], 0.0)
            # warm PSUM rides the ps ring (slot recycled by the 3rd pair)
            wps = psp.tile([64, 64], f32, tag="ps", name="wps")
            for wi in range(nwarm):
                nc.tensor.matmul(
                    wps[:], warm[:, :64], warm[:, :64],
                    start=(wi == 0), stop=(wi == nwarm - 1),
                )
            # weights: even-block groups first (they gate the first blocks);
            # odd groups ride after the first input supertiles — the serial
            # DMA engine then has the first blocks' inputs landed before
            # the PE needs the odd-group bands (block 3, ~6.5us).
            nc.sync.dma_start(wt[:, : _NGE, :, :], w_d[:, : _NGE, :, :])
            order = [_NBLOCKS - 1] + list(range(_NBLOCKS - 1))
            # supertile table: (pos_start, nsup, first block)
            stiles = []
            s0 = 0
            for n in supers:
                stiles.append((s0, n, order[s0]))
                s0 += n
            sup_of = {p: j for j, (p, _, _) in enumerate(stiles)}
            xts = {}

            def load_supertile(j):
                _, nsup, b0 = stiles[j]
                g0 = b0 * _M
                xts[j] = inp.tile([_KIN, _SB, _SLOT], f8, tag="xt",
                                  name="xt")
                # one DMA: nsup blocks of 128 rows, 112 rows apart
                dst = xts[j][:, :nsup, : _WP]
                src1 = x_d[g0 : g0 + _KIN, :]
                src = dataclasses.replace(
                    src1,
                    ap=[list(src1.ap[0]), [_M * _WP, nsup], [1, _WP]],
                )
                nc.sync.dma_start(dst, src)

            sup_base = 0
            sup_j = 0
            xt = None
            ps = None
            ots = {}
            for pos, b in enumerate(order):
                if pos in sup_of:
                    sup_j = sup_of[pos]
                    sup_base = pos
                    if sup_j == 0:
                        # prefetch depth 2: supertile j+1's input DMA is
                        # issued at supertile j's start, so output-store
                        # SemWaits parked at the SP queue head never
                        # delay the input the PE needs next.
                        load_supertile(0)
                        load_supertile(1)
                    elif sup_j + 1 < len(stiles):
                        load_supertile(sup_j + 1)
                    xt = xts.pop(sup_j)
                    if sup_j == 1:
                        nc.sync.dma_start(
                            wt[:, _NGE :, :, :], w_d[:, _NGE :, :, :]
                        )
                i = pos - sup_base
                slot = xt[:, i, :]
                odd = _is_odd_block(b)
                # mirror pre-sums: s1 on DVE everywhere; s2/s3 mostly on
                # Pool for 7-DR blocks (InstTensorScalarPtr: in0*1 + in1).
                nc.vector.tensor_add(
                    xt[:, i, _S1 : _S1 + _W],
                    xt[:, i, 0 : _W],
                    xt[:, i, 16 : 16 + _W],
                )
                if odd:
                    # s2 is column-split between DVE (z cols) and Pool
                    # (rest + all of s3, via InstTensorTensor — the only
                    # elementwise op GPSIMD codegen accepts) so both
                    # engines stay just under the 15-DR pair budget.
                    z = 288
                    nc.vector.tensor_add(
                        xt[:, i, _S2 : _S2 + z],
                        xt[:, i, 1 : 1 + z],
                        xt[:, i, 15 : 15 + z],
                    )
                    nc.gpsimd.tensor_add(
                        xt[:, i, _S2 + z : _S2 + _W],
                        xt[:, i, 1 + z : 1 + _W],
                        xt[:, i, 15 + z : 15 + _W],
                    )
                    nc.gpsimd.tensor_add(
                        xt[:, i, _S3 : _S3 + _W],
                        xt[:, i, 2 : 2 + _W],
                        xt[:, i, 14 : 14 + _W],
                    )
                # PSUM hazards are tile-granular: a block whose tile was
                # already partially copied would stall on that copy, so
                # the tail blocks (54, 55) get their own tiles.
                if b % 2 == 0 or b == _NBLOCKS - 1:
                    ps = psp.tile([_M, 2, _W], f32, tag="ps")
                half = b % 2 if b != _NBLOCKS - 1 else 0
                pdim = list(slot.ap[0])
                pairs = _PAIRS_ODD if odd else _PAIRS_EVEN
                goff = _NGE if odd else 0
                ng = len(pairs)
                gs, gn = _store_group(b)
                if b == gs:
                    ots[gs] = outp.tile(
                        [_M, gn, _W], f16, tag="ot", name="ot"
                    )
                ot = ots[gs]
                if b == 55:
                    # last computed block: column-halved matmul groups so
                    # the first half's copy overlaps the second half's
                    # matmuls — shortens the drain tail. Each half gets
                    # its own PSUM tile (tile-granular hazards). Copies
                    # spread over Act/Pool (Pool has no ack-return delay
                    # in its sem chain); blocks 54+55 share ONE store.
                    hw_ = _W // 2
                    for ch in (0, 1):
                        c0 = ch * hw_
                        psh = psp.tile([_M, 2, _W], f32, tag="ps",
                                       name="ps")
                        for g, (a, bb) in enumerate(pairs):
                            rhs = dataclasses.replace(
                                slot,
                                offset=slot.offset + a + c0,
                                ap=[pdim, [bb - a, 2], [1, hw_]],
                            )
                            nc.tensor.matmul(
                                psh[:, 0, c0 : c0 + hw_],
                                wt[:, goff + g, :, :], rhs,
                                start=(g == 0), stop=(g == ng - 1),
                                perf_mode=DR,
                            )
                        if ch == 0:
                            nc.scalar.mul(
                                ot[:, 1, c0 : c0 + hw_],
                                psh[:, 0, c0 : c0 + hw_], 1.0 / _WSCALE,
                            )
                        else:
                            # GPSIMD cannot read PSUM; DVE (idle by now)
                            # runs the final half in parallel with Act's.
                            nc.vector.tensor_scalar_mul(
                                ot[:, 1, c0 : c0 + hw_],
                                psh[:, 0, c0 : c0 + hw_], 1.0 / _WSCALE,
                            )
                    od1 = o_d[54 * _M : 55 * _M, :]
                    odst = dataclasses.replace(
                        od1,
                        ap=[list(od1.ap[0]), [_M * _W, 2], [1, _W]],
                    )
                    nc.sync.dma_start(odst, ot[:, :2, :])
                    continue
                for g, (a, bb) in enumerate(pairs):
                    rhs = dataclasses.replace(
                        slot,
                        offset=slot.offset + a,
                        ap=[pdim, [bb - a, 2], [1, _W]],
                    )
                    nc.tensor.matmul(
                        ps[:, half, :], wt[:, goff + g, :, :], rhs,
                        start=(g == 0), stop=(g == ng - 1),
                        perf_mode=DR,
                    )
                # descale-copies: batched per block pair on Activation;
                # singles for the tail blocks.
                if b == _NBLOCKS - 1:
                    nv = _NSTART - b * _M
                    nc.scalar.mul(
                        ot[:nv, 0, :], ps[:nv, 0, :], 1.0 / _WSCALE
                    )
                elif b == 54:
                    # DVE is idle by now; keeps Act free for block 55's
                    # first-half copy
                    nc.vector.tensor_scalar_mul(
                        ot[:, 0, :], ps[:, half, :], 1.0 / _WSCALE
                    )
                elif b % 2 == 1:
                    c0 = b - 1 - gs
                    nc.scalar.mul(
                        ot[:, c0 : c0 + 2, :], ps[:, :, :], 1.0 / _WSCALE
                    )
                if b == gs + gn - 1:
                    nv = min(_NSTART - gs * _M, gn * _M)
                    od1 = o_d[gs * _M : gs * _M + min(nv, _M), :]
                    if gn == 1:
                        nc.sync.dma_start(od1[:nv, :], ot[:nv, 0, :])
                    else:
                        osrc = ot[:, :gn, :]
                        odst = dataclasses.replace(
                            od1,
                            ap=[list(od1.ap[0]), [_M * _W, gn], [1, _W]],
                        )
                        nc.sync.dma_start(odst, osrc)
                    del ots[gs]
    nc.compile()
    return nc


def _get_program():
    if not _NC_CACHE:
        _NC_CACHE.append(_build_program())
    return _NC_CACHE[0]


def kernel(images: np.ndarray) -> np.ndarray:
    import ml_dtypes
    from concourse.bass_utils import run_bass_kernel_spmd

    f8 = ml_dtypes.float8_e4m3
    images = np.asarray(images, dtype=np.float32)
    padded = np.pad(
        images, ((0, 0), (0, 0), (_RADIUS, _RADIUS), (_RADIUS, _RADIUS)),
        mode="reflect",
    )
    shards = padded.reshape(_NCORES, _PLANES * _HP, _WP)
    xs = np.zeros((_NCORES, _XROWS, _WP), dtype=f8)
    xs[:, : _PLANES * _HP, :] = _quantize_diffuse(shards, f8)
    w = _banded_weights()
    nc = _get_program()
    in_maps = [{"x": xs[c], "w": w} for c in range(_NCORES)]
    res = run_bass_kernel_spmd(nc, in_maps, list(range(_NCORES)))
    out = np.stack(
        [np.asarray(res.results[c]["o"]) for c in range(_NCORES)], axis=0
    )
    out = out[:, : _PLANES * _HP, :].astype(np.float32)
    out = out.reshape(_NCORES, _PLANES, _HP, _W)[:, :, : _H, :]
    return np.ascontiguousarray(
        out.reshape(_B, _C, _H, _W).astype(np.float32)
    )
